# revision 35
# baseline (speedup 1.0000x reference)
"""Trainium2 Bass kernel for 5 iterated 3x3 cross-diffusion convs (NCHW).

Math: one step is x <- 0.25*(A x + x B) with A,B the 1024x1024 finite
tridiagonal shift-sum matrices (zero-pad semantics). A and B commute, so
    out = (1/4^5) * sum_k C(5,k) A^k x B^(5-k).
Horizontal powers H_j = x B^j (j<=4) are built with 4 shifted-add passes
split across DVE and GpSimd (exact finite-B via zero guard columns); the
B^5 term reads H_4 at +-1 column offsets. Vertical powers are float32r PE
matmuls with host-precomputed banded weight blocks, accumulated in PSUM.

Layout: each 1024-row image is processed as 9 overlapping row windows of
128 (partition dim), each producing 118 valid output rows -> no halo
fixup matmuls. Data parallel over batch: 4 images per NeuronCore x 8.
"""

import math
import os
from contextlib import ExitStack

import numpy as np

import concourse.tile as tile
from concourse import bacc, mybir
from concourse.bass_utils import run_bass_kernel_spmd

N_CORES = 8
IMGS_PER_CORE = 4
H_IMG = 1024
W_IMG = 1024
VALID = 118          # valid output rows per block
HALO = 5             # stencil reach after 5 steps
NBLK = 9             # ceil(1024/118)
G = 8                # zero guard columns on each side
PITCH = G + W_IMG + G
NSTEP = 5
GROUPS = [(0, 1, 2), (3, 4, 5), (6, 7, 8)]
F32R = mybir.dt.float32r
F32 = mybir.dt.float32

_cache = {}

LAST_EXEC_NS = None
LAST_RESULT = None


def _build_weights() -> np.ndarray:
    """w[p, k*9+T, m] = C(5,k)/1024 * (A^k)[118T+m, win_start(T)+p]."""
    A = np.diag(np.ones(H_IMG - 1), 1) + np.diag(np.ones(H_IMG - 1), -1)
    Apow = [np.eye(H_IMG)]
    for _ in range(NSTEP):
        Apow.append(Apow[-1] @ A)
    # free dim padded 118 -> 128 so each weight slice starts 512B-aligned
    # (f32r matmul weight APs at unaligned offsets are fatal on HW)
    w = np.zeros((128, NBLK * 6, 128), np.float32)
    for T in range(NBLK):
        r0 = VALID * T
        nv = min(VALID, H_IMG - r0)
        ws = min(max(r0 - HALO, 0), H_IMG - 128)
        for k in range(6):
            c = math.comb(NSTEP, k) / 4.0**NSTEP
            blk = np.zeros((128, VALID))
            for p in range(128):
                blk[p, :nv] = Apow[k][r0:r0 + nv, ws + p]
            w[:, k * NBLK + T, :VALID] = c * blk
    return w


def _win_start(T: int) -> int:
    return min(max(VALID * T - HALO, 0), H_IMG - 128)


def _build_program():
    nc = bacc.Bacc(
        "TRN2", target_bir_lowering=False, debug=False, num_devices=N_CORES
    )
    x_ap = nc.dram_tensor(
        "x", [IMGS_PER_CORE, H_IMG, W_IMG], F32R, kind="ExternalInput"
    ).ap()
    w_ap = nc.dram_tensor(
        "w", [128, NBLK * 6, 128], F32R, kind="ExternalInput"
    ).ap()
    y_ap = nc.dram_tensor(
        "y", [IMGS_PER_CORE, H_IMG, W_IMG], F32, kind="ExternalOutput"
    ).ap()

    with tile.TileContext(nc) as tc, ExitStack() as ctx:
        wpool = ctx.enter_context(tc.tile_pool(name="wpool", bufs=1))
        xpool = ctx.enter_context(tc.tile_pool(name="xpool", bufs=3))
        hpool = ctx.enter_context(tc.tile_pool(name="hpool", bufs=1))
        pspool = ctx.enter_context(tc.tile_pool(name="ps", bufs=4, space="PSUM"))
        stpool = ctx.enter_context(tc.tile_pool(name="st", bufs=6))

        wt = wpool.tile([128, NBLK * 6, 128], F32R)
        # load weights per k, descending (stream order) so the first
        # matmul stream is not gated on the full 3.5 MB weight load
        for k in range(NSTEP, -1, -1):
            nc.scalar.dma_start(
                wt[:, k * NBLK:(k + 1) * NBLK, :],
                w_ap[:, k * NBLK:(k + 1) * NBLK, :],
            )

        # Persistent horizontal-power tiles, two sets alternating by group
        # parity so group g+1's H-chain overlaps group g's matmuls; memset
        # once zeroes the guards. H_5 is never materialized: the k=0 term
        # reads H_4 at column offsets +-1 instead (B^5 = B * B^4).
        hsets = [
            [
                hpool.tile([128, 3, PITCH], F32R, name=f"h{s}_{j}")
                for j in range(NSTEP - 1)
            ]
            for s in range(2)
        ]
        for hset in hsets:
            for h in hset:
                nc.gpsimd.memset(h[:, :, 0:G].bitcast(F32), 0.0)
                nc.gpsimd.memset(h[:, :, G + W_IMG:PITCH].bitcast(F32), 0.0)

        gi = 0
        for img in range(IMGS_PER_CORE):
            for grp in GROUPS:
                hts = hsets[gi % 2]
                gi += 1
                xt = xpool.tile([128, 3, PITCH], F32R, name="xt", tag="xt")
                # zero guard columns (tile buffers are recycled)
                nc.vector.memset(xt[:, :, 0:G].bitcast(F32), 0.0)
                nc.gpsimd.memset(xt[:, :, G + W_IMG:PITCH].bitcast(F32), 0.0)
                for i, T in enumerate(grp):
                    ws = _win_start(T)
                    nc.sync.dma_start(
                        xt[:, i, G:G + W_IMG], x_ap[img, ws:ws + 128, :]
                    )

                # H-chain: H_j = H_{j-1} shifted-left + shifted-right.
                # Split per step: DVE does blocks 0-1, GpSimd block 2.
                nb = len(grp)
                nd = max(1, (2 * nb) // 3)
                prev = xt
                for j in range(NSTEP - 1):
                    h = hts[j]
                    nc.vector.tensor_add(
                        h[:, 0:nd, G:G + W_IMG],
                        prev[:, 0:nd, G - 1:G - 1 + W_IMG],
                        prev[:, 0:nd, G + 1:G + 1 + W_IMG],
                    )
                    if nd < nb:
                        nc.gpsimd.tensor_add(
                            h[:, nd:nb, G:G + W_IMG],
                            prev[:, nd:nb, G - 1:G - 1 + W_IMG],
                            prev[:, nd:nb, G + 1:G + 1 + W_IMG],
                        )
                    prev = h

                # PE: psum[T] += W[T,k]^T @ (H_{5-k}[T] shifted by d)
                streams = [
                    (5, None, 0), (4, 0, 0), (3, 1, 0), (2, 2, 0),
                    (1, 3, 0), (0, 3, -1), (0, 3, 1),
                ]
                pss = {}
                for kk, (k, hj, d) in enumerate(streams[:-1]):
                    rhs = xt if hj is None else hts[hj]
                    for i, T in enumerate(grp):
                        if kk == 0:
                            pss[T] = pspool.tile(
                                [VALID, 1024], F32, name="pst", tag="pst"
                            )
                        for h2 in range(2):
                            o = G + d + h2 * 512
                            nc.tensor.matmul(
                                pss[T][:, h2 * 512:(h2 + 1) * 512],
                                wt[:, k * NBLK + T, 0:VALID],
                                rhs[:, i, o:o + 512],
                                start=(kk == 0),
                                stop=False,
                            )
                k, hj, d = streams[-1]
                for i, T in enumerate(grp):
                    rhs = xt if hj is None else hts[hj]
                    for h2 in range(2):
                        o = G + d + h2 * 512
                        nc.tensor.matmul(
                            pss[T][:, h2 * 512:(h2 + 1) * 512],
                            wt[:, k * NBLK + T, 0:VALID],
                            rhs[:, i, o:o + 512],
                            start=False,
                            stop=True,
                        )
                    nv = min(VALID, H_IMG - VALID * T)
                    st = stpool.tile([VALID, 1024], F32, name="stt", tag="stt")
                    nc.scalar.copy(st[:], pss[T][:])
                    nc.scalar.dma_start(
                        y_ap[img, VALID * T:VALID * T + nv, :], st[0:nv, :]
                    )

    nc.compile()
    return nc


def kernel(x, weight=None, **_unused) -> np.ndarray:
    global LAST_EXEC_NS, LAST_RESULT
    x = np.ascontiguousarray(np.asarray(x), dtype=np.float32)
    assert x.shape == (N_CORES * IMGS_PER_CORE, 1, H_IMG, W_IMG), x.shape

    if "nc" not in _cache:
        _cache["w"] = _build_weights()
        _cache["nc"] = _build_program()
    nc = _cache["nc"]
    wts = _cache["w"]

    xs = x.reshape(N_CORES * IMGS_PER_CORE, H_IMG, W_IMG)
    in_maps = [
        {"x": np.ascontiguousarray(xs[IMGS_PER_CORE * c:IMGS_PER_CORE * (c + 1)]),
         "w": wts}
        for c in range(N_CORES)
    ]
    trace = bool(int(os.environ.get("KERNEL_TRACE", "0")))
    try:
        res = run_bass_kernel_spmd(
            nc, in_maps, core_ids=list(range(N_CORES)), trace=trace
        )
    except ModuleNotFoundError:
        os.environ["BASS_NEVER_TRACE"] = "1"
        res = run_bass_kernel_spmd(
            nc, in_maps, core_ids=list(range(N_CORES)), trace=False
        )
    LAST_RESULT = res
    LAST_EXEC_NS = res.exec_time_ns
    out = np.stack([res.results[c]["y"] for c in range(N_CORES)], axis=0)
    return out.reshape(N_CORES * IMGS_PER_CORE, 1, H_IMG, W_IMG)


# revision 36
# speedup vs baseline: 1.0066x; 1.0066x over previous
"""Trainium2 Bass kernel for 5 iterated 3x3 cross-diffusion convs (NCHW).

Math: one step is x <- 0.25*(A x + x B) with A,B the 1024x1024 finite
tridiagonal shift-sum matrices (zero-pad semantics). A and B commute, so
    out = (1/4^5) * sum_k C(5,k) A^k x B^(5-k).
Horizontal powers H_j = x B^j (j<=4) are built with 4 shifted-add passes
split across DVE and GpSimd (exact finite-B via zero guard columns); the
B^5 term reads H_4 at +-1 column offsets. Vertical powers are float32r PE
matmuls with host-precomputed banded weight blocks, accumulated in PSUM.

Layout: each 1024-row image is processed as 9 overlapping row windows of
128 (partition dim), each producing 118 valid output rows -> no halo
fixup matmuls. Data parallel over batch: 4 images per NeuronCore x 8.
"""

import math
import os
from contextlib import ExitStack

import numpy as np

import concourse.tile as tile
from concourse import bacc, mybir
from concourse.bass_utils import run_bass_kernel_spmd

N_CORES = 8
IMGS_PER_CORE = 4
H_IMG = 1024
W_IMG = 1024
VALID = 118          # valid output rows per block
HALO = 5             # stencil reach after 5 steps
NBLK = 9             # ceil(1024/118)
G = 8                # zero guard columns on each side
PITCH = G + W_IMG + G
NSTEP = 5
GROUPS = [(0, 1, 2), (3, 4, 5), (6, 7, 8)]
F32R = mybir.dt.float32r
F32 = mybir.dt.float32

_cache = {}

LAST_EXEC_NS = None
LAST_RESULT = None


def _build_weights() -> np.ndarray:
    """w[p, k*9+T, m] = C(5,k)/1024 * (A^k)[118T+m, win_start(T)+p]."""
    A = np.diag(np.ones(H_IMG - 1), 1) + np.diag(np.ones(H_IMG - 1), -1)
    Apow = [np.eye(H_IMG)]
    for _ in range(NSTEP):
        Apow.append(Apow[-1] @ A)
    # free dim padded 118 -> 128 so each weight slice starts 512B-aligned
    # (f32r matmul weight APs at unaligned offsets are fatal on HW)
    w = np.zeros((128, NBLK * 6, 128), np.float32)
    for T in range(NBLK):
        r0 = VALID * T
        nv = min(VALID, H_IMG - r0)
        ws = min(max(r0 - HALO, 0), H_IMG - 128)
        for k in range(6):
            c = math.comb(NSTEP, k) / 4.0**NSTEP
            blk = np.zeros((128, VALID))
            for p in range(128):
                blk[p, :nv] = Apow[k][r0:r0 + nv, ws + p]
            w[:, k * NBLK + T, :VALID] = c * blk
    return w


def _win_start(T: int) -> int:
    return min(max(VALID * T - HALO, 0), H_IMG - 128)


def _build_program():
    nc = bacc.Bacc(
        "TRN2", target_bir_lowering=False, debug=False, num_devices=N_CORES
    )
    x_ap = nc.dram_tensor(
        "x", [IMGS_PER_CORE, H_IMG, W_IMG], F32R, kind="ExternalInput"
    ).ap()
    w_ap = nc.dram_tensor(
        "w", [128, NBLK * 6, 128], F32R, kind="ExternalInput"
    ).ap()
    y_ap = nc.dram_tensor(
        "y", [IMGS_PER_CORE, H_IMG, W_IMG], F32, kind="ExternalOutput"
    ).ap()

    with tile.TileContext(nc) as tc, ExitStack() as ctx:
        wpool = ctx.enter_context(tc.tile_pool(name="wpool", bufs=1))
        xpool = ctx.enter_context(tc.tile_pool(name="xpool", bufs=3))
        hpool = ctx.enter_context(tc.tile_pool(name="hpool", bufs=1))
        pspool = ctx.enter_context(tc.tile_pool(name="ps", bufs=4, space="PSUM"))
        stpool = ctx.enter_context(tc.tile_pool(name="st", bufs=6))

        wt = wpool.tile([128, NBLK * 6, 128], F32R)
        # load weights per k, descending (stream order) so the first
        # matmul stream is not gated on the full 3.5 MB weight load
        for k in range(NSTEP, -1, -1):
            nc.scalar.dma_start(
                wt[:, k * NBLK:(k + 1) * NBLK, :],
                w_ap[:, k * NBLK:(k + 1) * NBLK, :],
            )

        # Persistent horizontal-power tiles, two sets alternating by group
        # parity so group g+1's H-chain overlaps group g's matmuls; memset
        # once zeroes the guards. H_5 is never materialized: the k=0 term
        # reads H_4 at column offsets +-1 instead (B^5 = B * B^4).
        hsets = [
            [
                hpool.tile([128, 3, PITCH], F32R, name=f"h{s}_{j}")
                for j in range(NSTEP - 1)
            ]
            for s in range(2)
        ]
        for hset in hsets:
            for h in hset:
                nc.gpsimd.memset(h[:, :, 0:G].bitcast(F32), 0.0)
                nc.gpsimd.memset(h[:, :, G + W_IMG:PITCH].bitcast(F32), 0.0)

        gi = 0
        for img in range(IMGS_PER_CORE):
            for grp in GROUPS:
                hts = hsets[gi % 2]
                gi += 1
                xt = xpool.tile([128, 3, PITCH], F32R, name="xt", tag="xt")
                # zero guard columns (tile buffers are recycled)
                nc.vector.memset(xt[:, :, 0:G].bitcast(F32), 0.0)
                nc.gpsimd.memset(xt[:, :, G + W_IMG:PITCH].bitcast(F32), 0.0)
                for i, T in enumerate(grp):
                    ws = _win_start(T)
                    nc.sync.dma_start(
                        xt[:, i, G:G + W_IMG], x_ap[img, ws:ws + 128, :]
                    )

                # H-chain: H_j = H_{j-1} shifted-left + shifted-right.
                # Split per step: DVE does blocks 0-1, GpSimd block 2.
                nb = len(grp)
                nd = max(1, (2 * nb) // 3)
                prev = xt
                for j in range(NSTEP - 1):
                    h = hts[j]
                    if gi == 1:
                        # first group: per-block chain ops so the earliest
                        # matmul streams unblock per block (pipeline fill)
                        for i in range(nd):
                            nc.vector.tensor_add(
                                h[:, i:i + 1, G:G + W_IMG],
                                prev[:, i:i + 1, G - 1:G - 1 + W_IMG],
                                prev[:, i:i + 1, G + 1:G + 1 + W_IMG],
                            )
                    else:
                        nc.vector.tensor_add(
                            h[:, 0:nd, G:G + W_IMG],
                            prev[:, 0:nd, G - 1:G - 1 + W_IMG],
                            prev[:, 0:nd, G + 1:G + 1 + W_IMG],
                        )
                    if nd < nb:
                        nc.gpsimd.tensor_add(
                            h[:, nd:nb, G:G + W_IMG],
                            prev[:, nd:nb, G - 1:G - 1 + W_IMG],
                            prev[:, nd:nb, G + 1:G + 1 + W_IMG],
                        )
                    prev = h

                # PE: psum[T] += W[T,k]^T @ (H_{5-k}[T] shifted by d)
                streams = [
                    (5, None, 0), (4, 0, 0), (3, 1, 0), (2, 2, 0),
                    (1, 3, 0), (0, 3, -1), (0, 3, 1),
                ]
                pss = {}
                for kk, (k, hj, d) in enumerate(streams[:-1]):
                    rhs = xt if hj is None else hts[hj]
                    for i, T in enumerate(grp):
                        if kk == 0:
                            pss[T] = pspool.tile(
                                [VALID, 1024], F32, name="pst", tag="pst"
                            )
                        for h2 in range(2):
                            o = G + d + h2 * 512
                            nc.tensor.matmul(
                                pss[T][:, h2 * 512:(h2 + 1) * 512],
                                wt[:, k * NBLK + T, 0:VALID],
                                rhs[:, i, o:o + 512],
                                start=(kk == 0),
                                stop=False,
                            )
                k, hj, d = streams[-1]
                for i, T in enumerate(grp):
                    rhs = xt if hj is None else hts[hj]
                    for h2 in range(2):
                        o = G + d + h2 * 512
                        nc.tensor.matmul(
                            pss[T][:, h2 * 512:(h2 + 1) * 512],
                            wt[:, k * NBLK + T, 0:VALID],
                            rhs[:, i, o:o + 512],
                            start=False,
                            stop=True,
                        )
                    nv = min(VALID, H_IMG - VALID * T)
                    st = stpool.tile([VALID, 1024], F32, name="stt", tag="stt")
                    if gi == IMGS_PER_CORE * len(GROUPS) and i > 0:
                        # kernel tail: DVE is idle, ACT still issues the
                        # final output DMAs -- split the last drains
                        nc.vector.tensor_copy(st[:], pss[T][:])
                    else:
                        nc.scalar.copy(st[:], pss[T][:])
                    nc.scalar.dma_start(
                        y_ap[img, VALID * T:VALID * T + nv, :], st[0:nv, :]
                    )

    nc.compile()
    return nc


def kernel(x, weight=None, **_unused) -> np.ndarray:
    global LAST_EXEC_NS, LAST_RESULT
    x = np.ascontiguousarray(np.asarray(x), dtype=np.float32)
    assert x.shape == (N_CORES * IMGS_PER_CORE, 1, H_IMG, W_IMG), x.shape

    if "nc" not in _cache:
        _cache["w"] = _build_weights()
        _cache["nc"] = _build_program()
    nc = _cache["nc"]
    wts = _cache["w"]

    xs = x.reshape(N_CORES * IMGS_PER_CORE, H_IMG, W_IMG)
    in_maps = [
        {"x": np.ascontiguousarray(xs[IMGS_PER_CORE * c:IMGS_PER_CORE * (c + 1)]),
         "w": wts}
        for c in range(N_CORES)
    ]
    trace = bool(int(os.environ.get("KERNEL_TRACE", "0")))
    try:
        res = run_bass_kernel_spmd(
            nc, in_maps, core_ids=list(range(N_CORES)), trace=trace
        )
    except ModuleNotFoundError:
        os.environ["BASS_NEVER_TRACE"] = "1"
        res = run_bass_kernel_spmd(
            nc, in_maps, core_ids=list(range(N_CORES)), trace=False
        )
    LAST_RESULT = res
    LAST_EXEC_NS = res.exec_time_ns
    out = np.stack([res.results[c]["y"] for c in range(N_CORES)], axis=0)
    return out.reshape(N_CORES * IMGS_PER_CORE, 1, H_IMG, W_IMG)


# revision 37
# speedup vs baseline: 1.0100x; 1.0034x over previous
"""Trainium2 Bass kernel for 5 iterated 3x3 cross-diffusion convs (NCHW).

Math: one step is x <- 0.25*(A x + x B) with A,B the 1024x1024 finite
tridiagonal shift-sum matrices (zero-pad semantics). A and B commute, so
    out = (1/4^5) * sum_k C(5,k) A^k x B^(5-k).
Horizontal powers H_j = x B^j (j<=4) are built with 4 shifted-add passes
split across DVE and GpSimd (exact finite-B via zero guard columns); the
B^5 term reads H_4 at +-1 column offsets. Vertical powers are float32r PE
matmuls with host-precomputed banded weight blocks, accumulated in PSUM.

Layout: each 1024-row image is processed as 9 overlapping row windows of
128 (partition dim), each producing 118 valid output rows -> no halo
fixup matmuls. Data parallel over batch: 4 images per NeuronCore x 8.
"""

import math
import os
from contextlib import ExitStack

import numpy as np

import concourse.tile as tile
from concourse import bacc, mybir
from concourse.bass_utils import run_bass_kernel_spmd

N_CORES = 8
IMGS_PER_CORE = 4
H_IMG = 1024
W_IMG = 1024
VALID = 118          # valid output rows per block
HALO = 5             # stencil reach after 5 steps
NBLK = 9             # ceil(1024/118)
G = 8                # zero guard columns on each side
PITCH = G + W_IMG + G
NSTEP = 5
GROUPS = [(0, 1, 2), (3, 4, 5), (6, 7, 8)]
F32R = mybir.dt.float32r
F32 = mybir.dt.float32

_cache = {}

LAST_EXEC_NS = None
LAST_RESULT = None


def _build_weights() -> np.ndarray:
    """w[p, k*9+T, m] = C(5,k)/1024 * (A^k)[118T+m, win_start(T)+p]."""
    A = np.diag(np.ones(H_IMG - 1), 1) + np.diag(np.ones(H_IMG - 1), -1)
    Apow = [np.eye(H_IMG)]
    for _ in range(NSTEP):
        Apow.append(Apow[-1] @ A)
    # free dim padded 118 -> 128 so each weight slice starts 512B-aligned
    # (f32r matmul weight APs at unaligned offsets are fatal on HW)
    w = np.zeros((128, NBLK * 6, 128), np.float32)
    for T in range(NBLK):
        r0 = VALID * T
        nv = min(VALID, H_IMG - r0)
        ws = min(max(r0 - HALO, 0), H_IMG - 128)
        for k in range(6):
            c = math.comb(NSTEP, k) / 4.0**NSTEP
            blk = np.zeros((128, VALID))
            for p in range(128):
                blk[p, :nv] = Apow[k][r0:r0 + nv, ws + p]
            w[:, k * NBLK + T, :VALID] = c * blk
    return w


def _win_start(T: int) -> int:
    return min(max(VALID * T - HALO, 0), H_IMG - 128)


def _build_program():
    nc = bacc.Bacc(
        "TRN2", target_bir_lowering=False, debug=False, num_devices=N_CORES
    )
    x_ap = nc.dram_tensor(
        "x", [IMGS_PER_CORE, H_IMG, W_IMG], F32R, kind="ExternalInput"
    ).ap()
    w_ap = nc.dram_tensor(
        "w", [128, NBLK * 6, 128], F32R, kind="ExternalInput"
    ).ap()
    y_ap = nc.dram_tensor(
        "y", [IMGS_PER_CORE, H_IMG, W_IMG], F32, kind="ExternalOutput"
    ).ap()

    with tile.TileContext(nc) as tc, ExitStack() as ctx:
        wpool = ctx.enter_context(tc.tile_pool(name="wpool", bufs=1))
        xpool = ctx.enter_context(tc.tile_pool(name="xpool", bufs=3))
        hpool = ctx.enter_context(tc.tile_pool(name="hpool", bufs=1))
        pspool = ctx.enter_context(tc.tile_pool(name="ps", bufs=8, space="PSUM"))
        stpool = ctx.enter_context(tc.tile_pool(name="st", bufs=6))

        wt = wpool.tile([128, NBLK * 6, 128], F32R)
        # load weights per k, descending (stream order) so the first
        # matmul stream is not gated on the full 3.5 MB weight load
        for k in range(NSTEP, -1, -1):
            nc.scalar.dma_start(
                wt[:, k * NBLK:(k + 1) * NBLK, :],
                w_ap[:, k * NBLK:(k + 1) * NBLK, :],
            )

        # Persistent horizontal-power tiles, two sets alternating by group
        # parity so group g+1's H-chain overlaps group g's matmuls; memset
        # once zeroes the guards. H_5 is never materialized: the k=0 term
        # reads H_4 at column offsets +-1 instead (B^5 = B * B^4).
        hsets = [
            [
                hpool.tile([128, 3, PITCH], F32R, name=f"h{s}_{j}")
                for j in range(NSTEP - 1)
            ]
            for s in range(2)
        ]
        for hset in hsets:
            for h in hset:
                nc.gpsimd.memset(h[:, :, 0:G].bitcast(F32), 0.0)
                nc.gpsimd.memset(h[:, :, G + W_IMG:PITCH].bitcast(F32), 0.0)

        gi = 0
        for img in range(IMGS_PER_CORE):
            for grp in GROUPS:
                hts = hsets[gi % 2]
                gi += 1
                xt = xpool.tile([128, 3, PITCH], F32R, name="xt", tag="xt")
                # zero guard columns (tile buffers are recycled)
                nc.vector.memset(xt[:, :, 0:G].bitcast(F32), 0.0)
                nc.gpsimd.memset(xt[:, :, G + W_IMG:PITCH].bitcast(F32), 0.0)
                for i, T in enumerate(grp):
                    ws = _win_start(T)
                    nc.sync.dma_start(
                        xt[:, i, G:G + W_IMG], x_ap[img, ws:ws + 128, :]
                    )

                # H-chain: H_j = H_{j-1} shifted-left + shifted-right.
                # Split per step: DVE does blocks 0-1, GpSimd block 2.
                nb = len(grp)
                nd = max(1, (2 * nb) // 3)
                prev = xt
                for j in range(NSTEP - 1):
                    h = hts[j]
                    if gi == 1:
                        # first group: per-block chain ops so the earliest
                        # matmul streams unblock per block (pipeline fill)
                        for i in range(nd):
                            nc.vector.tensor_add(
                                h[:, i:i + 1, G:G + W_IMG],
                                prev[:, i:i + 1, G - 1:G - 1 + W_IMG],
                                prev[:, i:i + 1, G + 1:G + 1 + W_IMG],
                            )
                    else:
                        nc.vector.tensor_add(
                            h[:, 0:nd, G:G + W_IMG],
                            prev[:, 0:nd, G - 1:G - 1 + W_IMG],
                            prev[:, 0:nd, G + 1:G + 1 + W_IMG],
                        )
                    if nd < nb:
                        nc.gpsimd.tensor_add(
                            h[:, nd:nb, G:G + W_IMG],
                            prev[:, nd:nb, G - 1:G - 1 + W_IMG],
                            prev[:, nd:nb, G + 1:G + 1 + W_IMG],
                        )
                    prev = h

                # PE: psum[T] += W[T,k]^T @ (H_{5-k}[T] shifted by d)
                streams = [
                    (5, None, 0), (4, 0, 0), (3, 1, 0), (2, 2, 0),
                    (1, 3, 0), (0, 3, -1), (0, 3, 1),
                ]
                pss = {}
                for kk, (k, hj, d) in enumerate(streams[:-1]):
                    rhs = xt if hj is None else hts[hj]
                    for i, T in enumerate(grp):
                        for h2 in range(2):
                            if kk == 0:
                                pss[(T, h2)] = pspool.tile(
                                    [VALID, 512], F32, name="pst", tag="pst"
                                )
                            o = G + d + h2 * 512
                            nc.tensor.matmul(
                                pss[(T, h2)][:],
                                wt[:, k * NBLK + T, 0:VALID],
                                rhs[:, i, o:o + 512],
                                start=(kk == 0),
                                stop=False,
                            )
                k, hj, d = streams[-1]
                for i, T in enumerate(grp):
                    rhs = xt if hj is None else hts[hj]
                    nv = min(VALID, H_IMG - VALID * T)
                    st = stpool.tile([VALID, 1024], F32, name="stt", tag="stt")
                    last = gi == IMGS_PER_CORE * len(GROUPS)
                    for h2 in range(2):
                        o = G + d + h2 * 512
                        nc.tensor.matmul(
                            pss[(T, h2)][:],
                            wt[:, k * NBLK + T, 0:VALID],
                            rhs[:, i, o:o + 512],
                            start=False,
                            stop=True,
                        )
                        if last and i > 0:
                            nc.vector.tensor_copy(
                                st[:, h2 * 512:(h2 + 1) * 512], pss[(T, h2)][:]
                            )
                        else:
                            nc.scalar.copy(
                                st[:, h2 * 512:(h2 + 1) * 512], pss[(T, h2)][:]
                            )
                    nc.scalar.dma_start(
                        y_ap[img, VALID * T:VALID * T + nv, :], st[0:nv, :]
                    )

    nc.compile()
    return nc


def kernel(x, weight=None, **_unused) -> np.ndarray:
    global LAST_EXEC_NS, LAST_RESULT
    x = np.ascontiguousarray(np.asarray(x), dtype=np.float32)
    assert x.shape == (N_CORES * IMGS_PER_CORE, 1, H_IMG, W_IMG), x.shape

    if "nc" not in _cache:
        _cache["w"] = _build_weights()
        _cache["nc"] = _build_program()
    nc = _cache["nc"]
    wts = _cache["w"]

    xs = x.reshape(N_CORES * IMGS_PER_CORE, H_IMG, W_IMG)
    in_maps = [
        {"x": np.ascontiguousarray(xs[IMGS_PER_CORE * c:IMGS_PER_CORE * (c + 1)]),
         "w": wts}
        for c in range(N_CORES)
    ]
    trace = bool(int(os.environ.get("KERNEL_TRACE", "0")))
    try:
        res = run_bass_kernel_spmd(
            nc, in_maps, core_ids=list(range(N_CORES)), trace=trace
        )
    except ModuleNotFoundError:
        os.environ["BASS_NEVER_TRACE"] = "1"
        res = run_bass_kernel_spmd(
            nc, in_maps, core_ids=list(range(N_CORES)), trace=False
        )
    LAST_RESULT = res
    LAST_EXEC_NS = res.exec_time_ns
    out = np.stack([res.results[c]["y"] for c in range(N_CORES)], axis=0)
    return out.reshape(N_CORES * IMGS_PER_CORE, 1, H_IMG, W_IMG)
